# revision 36
# baseline (speedup 1.0000x reference)
"""Trainium2 Bass kernel for BlockChunkedActivityRoutedNet (v3, speculative).

Reference (B=4096, IN_F=4096, 8 chunks of 512, top-2 by mean|x| over the
whole batch, chunk Linears 512->512, concat -> final Linear 1024->4096):

    xr = x.reshape(B, 8, 512)
    activities = mean(|xr|, axis=(0, 2))
    i0, i1 = top2(activities)
    h = concat(xr[:, i0] @ Wc[i0] + bc[i0], xr[:, i1] @ Wc[i1] + bc[i1])
    out = h @ W_final + b_final

Distribution: data-parallel over batch across 8 cores (512 rows each).
Per-chunk |x| partial sums are AllGathered (32B per core) and summed
locally so every core computes identical top-2 routing.

Collective latency (entry skew + mesh floor) is hidden by SPECULATIVELY
running L1 for ALL 8 chunks while the AllGather is in flight.  hT for
every chunk is written to a DRAM table (row c*128+p holds that
partition's 4 d-tiles, 4KB rows); once routing lands, the two selected
chunks' hT is pulled back with 2 indirect-DMA row gathers (row =
sel*128 + p) and only L2 remains on the critical path.

DMA strategy: all SBUF-destined inputs are host-packed into [128, N]
matrices so every load is a single large dma_start with multi-KB
per-partition runs (one dma_start splits across all 16 SDMA engines).
Bulk loads ride the qSync HWDGE ring in priority order (x + first W
chunks first, W_final last); small loads, hT-table writes and output
writes ride the qAct ring so they never queue behind W_final.
"""

import numpy as np
import ml_dtypes

import concourse.bass as bass
import concourse.bass_isa as bass_isa
import concourse.bacc as bacc
import concourse.mybir as mybir
from concourse.tile import TileContext
from concourse.bass_utils import run_bass_kernel_spmd

dt = mybir.dt
P = 128

NUM_CHUNKS = 8
TOP_K = 2
IN_F = 4096
HID_F = 4096
OUT_F = 4096
B = 4096
CIN = IN_F // NUM_CHUNKS      # 512
COUT = HID_F // NUM_CHUNKS    # 512
N_CORES = 8
BS = B // N_CORES             # 512 rows per core

KT = CIN // P                 # 4 k-tiles per chunk (q groups)
DT_ = COUT // P               # 4 d-tiles per chunk
KF = TOP_K * DT_              # 8 k-tiles for the final matmul
OT = OUT_F // 512             # 8 output column tiles of 512
BT = BS // P                  # 4 batch tiles per core

XW = KT * BS                  # 2048: per-chunk x cols [q*512 + b]
WW = KT * COUT                # 2048: per-chunk w cols [q*512 + d]
CH_HEAD = 4                   # chunks whose weights load paired with x

_cache = {}


def _build():
    nc = bacc.Bacc(num_devices=N_CORES, name="chunk_routed_v3",
                   num_swdge_queues=4)

    # host-packed inputs: [128, N] matrices, per-partition-contiguous
    xh = nc.dram_tensor("xh", [P, NUM_CHUNKS * XW], dt.bfloat16,
                        kind="ExternalInput")
    wh = nc.dram_tensor("wh", [P, NUM_CHUNKS * WW], dt.bfloat16,
                        kind="ExternalInput")
    wfh = nc.dram_tensor("wfh", [P, KF * OUT_F], dt.bfloat16,
                         kind="ExternalInput")
    bct = nc.dram_tensor("bct", [COUT, NUM_CHUNKS], dt.float32,
                         kind="ExternalInput")
    bfb = nc.dram_tensor("bfb", [P, OUT_F], dt.float32,
                         kind="ExternalInput")
    out = nc.dram_tensor("out_shard", [BS, OUT_F], dt.float32,
                         kind="ExternalOutput")

    with TileContext(nc) as tc:
        with tc.tile_pool(name="consts", bufs=1) as consts, \
             tc.tile_pool(name="xl", bufs=1) as xl_pool, \
             tc.tile_pool(name="wkp", bufs=1) as wkp, \
             tc.tile_pool(name="wfp", bufs=8) as wfp, \
             tc.tile_pool(name="bias", bufs=1) as biasp, \
             tc.tile_pool(name="route", bufs=1) as route, \
             tc.tile_pool(name="hstage", bufs=3) as hstage, \
             tc.tile_pool(name="hselp", bufs=1) as hselp, \
             tc.tile_pool(name="outs", bufs=8) as outs, \
             tc.tile_pool(name="dram", bufs=1, space="DRAM") as dram:

            # ---------------- constants ----------------
            ones_col = consts.tile([P, 1], dt.float32)
            nc.vector.memset(ones_col[:], 1.0)
            ones_k1 = consts.tile([1, P], dt.float32)
            nc.vector.memset(ones_k1[:], 1.0)
            C_p = consts.tile([P, 1], dt.int32)
            nc.gpsimd.iota(C_p[:], pattern=[[1, 1]], base=0,
                           channel_multiplier=1)
            C_pf = consts.tile([P, 1], dt.float32)
            nc.vector.tensor_copy(C_pf[:], C_p[:])

            # ---------------- small loads (qAct ring) ----------------
            # b_final pre-broadcast on host: plain 2MB load, no PE/gpsimd
            bfin_bc = biasp.tile([P, OUT_F], dt.float32)
            nc.scalar.dma_start(bfin_bc[:], bfb[:])
            bias_sb = biasp.tile([P, DT_ * NUM_CHUNKS], dt.float32)
            # bias_sb[p, d*8+c] = b_chunks[c, 128d+p]
            for d in range(DT_):
                nc.scalar.dma_start(
                    bias_sb[:, d * NUM_CHUNKS:(d + 1) * NUM_CHUNKS],
                    bct[d * P:(d + 1) * P, :])

            # ---------------- bulk loads (qSync ring, priority FIFO) ----
            xls = [xl_pool.tile([P, XW], dt.bfloat16, tag=f"xl{c}",
                                name=f"xl{c}") for c in range(NUM_CHUNKS)]
            wks = [wkp.tile([P, WW], dt.bfloat16, tag=f"wk{c}",
                            name=f"wk{c}") for c in range(NUM_CHUNKS)]

            for c in range(NUM_CHUNKS):
                nc.sync.dma_start(xls[c][:], xh[:, c * XW:(c + 1) * XW])
            for c in range(NUM_CHUNKS):
                nc.sync.dma_start(wks[c][:], wh[:, c * WW:(c + 1) * WW])
            wf_t = [wfp.tile([P, OUT_F], dt.bfloat16, tag="wf",
                             name=f"wf{kf}") for kf in range(KF)]
            for kf in range(KF):
                nc.sync.dma_start(wf_t[kf][:],
                                  wfh[:, kf * OUT_F:(kf + 1) * OUT_F])

            # ---------------- activities ----------------
            # evens on scalar, odds on DVE (DVE is ~3x faster per reduce;
            # the last-arriving chunk 7 must clear quickly)
            actcol = route.tile([P, NUM_CHUNKS], dt.float32)
            scr = route.tile([P, XW], dt.bfloat16)   # Abs throwaway
            for c in range(NUM_CHUNKS):
                if c % 2 == 1:
                    nc.vector.tensor_reduce(
                        actcol[:, c:c + 1], xls[c][:],
                        axis=mybir.AxisListType.X, op=mybir.AluOpType.add,
                        apply_absolute_value=True)
                else:
                    nc.scalar.activation(
                        scr[:], xls[c][:],
                        mybir.ActivationFunctionType.Abs,
                        accum_out=actcol[:, c:c + 1])

            # collective buffers + hT table
            cc_in = dram.tile([1, NUM_CHUNKS], dt.float32)
            cc_out = dram.tile([1, N_CORES * NUM_CHUNKS], dt.float32)
            # hT table: row c*128+p = chunk c / partition p, cols = (d, b)
            hT_all = dram.tile([NUM_CHUNKS * P, DT_ * BS], dt.bfloat16)

            # ---------------- L1: speculative, all 8 chunks ----------------
            with tc.tile_pool(name="ps_pre", bufs=1, space="PSUM") as ps_pre, \
                 tc.tile_pool(name="ps_l1", bufs=2, space="PSUM") as ps_l1:

                def l1_chunk(c):
                    hsb = hstage.tile([P, DT_ * BS], dt.bfloat16, tag="hsb",
                                      name=f"hsb{c}")
                    for d in range(DT_):
                        ph = ps_l1.tile([P, BS], dt.float32, tag="ph",
                                        name=f"ph{c}_{d}")
                        for q in range(KT):
                            nc.tensor.matmul(
                                ph[:],
                                wks[c][:, q * COUT + d * P:
                                       q * COUT + (d + 1) * P],
                                xls[c][:, q * BS:(q + 1) * BS],
                                start=(q == 0), stop=(q == KT - 1))
                        nc.scalar.activation(
                            hsb[:, d * BS:(d + 1) * BS], ph[:],
                            mybir.ActivationFunctionType.Identity,
                            bias=bias_sb[:, d * NUM_CHUNKS + c:
                                         d * NUM_CHUNKS + c + 1])
                    nc.scalar.dma_start(
                        hT_all[c * P:(c + 1) * P, :], hsb[:])

                # activity partition-reduce + collective trigger first: the
                # PE idles until the chunk weights land anyway, and the
                # trigger must beat the CC-stream rendezvous floor.
                # act_l copy on DVE and cc_in on the (empty) SWDGE ring so
                # the trigger never queues behind evictions or hT writes.
                act_ps = ps_pre.tile([1, NUM_CHUNKS], dt.float32, tag="psa")
                nc.tensor.matmul(act_ps[:], ones_col[:], actcol[:],
                                 start=True, stop=True)
                act_l = route.tile([1, NUM_CHUNKS], dt.float32)
                nc.vector.tensor_copy(act_l[:], act_ps[:])
                nc.gpsimd.dma_start(cc_in[:], act_l[:])
                nc.gpsimd.collective_compute(
                    "AllGather", mybir.AluOpType.bypass,
                    replica_groups=[list(range(N_CORES))],
                    ins=[cc_in.opt()], outs=[cc_out.opt()])

                for c in range(NUM_CHUNKS):
                    l1_chunk(c)

                # ---------------- routing ----------------
                # ag_sb load on gpsimd only: on scalar/sync the Tile
                # scheduler hoists it before the tail L1 evictions and the
                # whole engine then blocks on the collective (priority
                # inversion that stalls L1's PSUM banks ~30us)
                ag_sb = route.tile([1, N_CORES * NUM_CHUNKS], dt.float32)
                nc.gpsimd.dma_start(ag_sb[:], cc_out[:])
                t32 = route.tile([1, 32], dt.float32)
                nc.vector.tensor_tensor(out=t32[:], in0=ag_sb[:, 0:32],
                                        in1=ag_sb[:, 32:64],
                                        op=mybir.AluOpType.add)
                t16 = route.tile([1, 16], dt.float32)
                nc.vector.tensor_tensor(out=t16[:], in0=t32[:, 0:16],
                                        in1=t32[:, 16:32],
                                        op=mybir.AluOpType.add)
                acts8 = route.tile([1, NUM_CHUNKS], dt.float32)
                nc.vector.tensor_tensor(out=acts8[:], in0=t16[:, 0:8],
                                        in1=t16[:, 8:16],
                                        op=mybir.AluOpType.add)
                maxv = route.tile([1, 8], dt.float32)
                nc.vector.max(maxv[:], acts8[:])
                maxi = route.tile([1, 8], dt.uint32)
                nc.vector.max_index(maxi[:], maxv[:], acts8[:])
                maxi_f = route.tile([1, 8], dt.float32)
                nc.vector.tensor_copy(maxi_f[:], maxi[:])

                # broadcast top-2 ids to all partitions with a K=1 matmul:
                # the PE is idle here and its next work (L2) needs routing
                # anyway; gpsimd.partition_broadcast costs a ~5us ucode
                # library switch on the critical path
                bc_ps = ps_pre.tile([P, TOP_K], dt.float32, tag="psb")
                nc.tensor.matmul(bc_ps[:], ones_k1[:], maxi_f[:, 0:TOP_K],
                                 start=True, stop=True)
                bc2f = route.tile([P, TOP_K], dt.float32)
                nc.vector.tensor_copy(bc2f[:], bc_ps[:])
                bc128 = route.tile([P, TOP_K], dt.float32)
                nc.vector.tensor_scalar_mul(bc128[:], bc2f[:], float(P))
                offW_f = route.tile([P, TOP_K], dt.float32)
                for s in range(TOP_K):
                    nc.vector.tensor_scalar(
                        offW_f[:, s:s + 1], C_pf[:], bc128[:, s:s + 1],
                        scalar2=None, op0=mybir.AluOpType.add)
                offW = route.tile([P, TOP_K], dt.int32)
                nc.vector.tensor_copy(offW[:], offW_f[:])

                # gather hT of the two selected chunks: row = sel*128 + p
                hsel = [hselp.tile([P, DT_ * BS], dt.bfloat16, tag=f"hs{s}",
                                   name=f"hs{s}") for s in range(TOP_K)]
                for s in range(TOP_K):
                    nc.gpsimd.indirect_dma_start(
                        out=hsel[s][:], out_offset=None,
                        in_=hT_all[:],
                        in_offset=bass.IndirectOffsetOnAxis(
                            ap=offW[:, s:s + 1], axis=0))

            # ---------------- L2: out = h @ W_final + b_final ----------
            with tc.tile_pool(name="ps_l2", bufs=8, space="PSUM") as ps_l2:
                for bt in range(BT):
                    po = [ps_l2.tile([P, 512], dt.float32, tag="po",
                                     name=f"po{bt}_{o}") for o in range(OT)]
                    for kf in range(KF):
                        s, d = divmod(kf, DT_)
                        lhsT = hsel[s][:, d * BS + bt * P:
                                       d * BS + (bt + 1) * P]
                        for o in range(OT):
                            nc.tensor.matmul(
                                po[o][:], lhsT,
                                wf_t[kf][:, o * 512:(o + 1) * 512],
                                start=(kf == 0), stop=(kf == KF - 1))
                    for o in range(OT):
                        osl = slice(o * 512, (o + 1) * 512)
                        ot_sb = outs.tile([P, 512], dt.float32, tag="ot",
                                          name=f"ot{bt}_{o}")
                        nc.vector.tensor_tensor(
                            out=ot_sb[:], in0=po[o][:], in1=bfin_bc[:, osl],
                            op=mybir.AluOpType.add)
                        # alternate rings: halves the serial write tail
                        eng = nc.scalar if o % 2 == 0 else nc.sync
                        eng.dma_start(
                            out[bt * P:(bt + 1) * P, osl], ot_sb[:])
    nc.compile()
    return nc


def _pack_inputs(x, W_chunks, W_final):
    """Host-side layout packing (pure layout, no computation)."""
    bf16 = ml_dtypes.bfloat16
    # per-core xh: [128, 8*2048], xh[p, c*2048 + q*512 + b] = x[b, 512c+4p+q]
    xhs = []
    for c0 in range(N_CORES):
        xs = np.asarray(x[c0 * BS:(c0 + 1) * BS], dtype=np.float32)  # [512, 4096]
        # [b, c, p, q] -> [p, c, q, b]
        xr = xs.reshape(BS, NUM_CHUNKS, P, KT).transpose(2, 1, 3, 0)
        xhs.append(np.ascontiguousarray(xr.reshape(P, NUM_CHUNKS * XW))
                   .astype(bf16))
    # wh[p, c*2048 + q*512 + d] = W[c, 4p+q, d]
    Wc = np.asarray(W_chunks, dtype=np.float32)
    wr = Wc.reshape(NUM_CHUNKS, P, KT, COUT).transpose(1, 0, 2, 3)
    wh = np.ascontiguousarray(wr.reshape(P, NUM_CHUNKS * WW)).astype(bf16)
    # wfh[p, kf*4096 + n] = W_final[128*kf + p, n]
    Wfin = np.asarray(W_final, dtype=np.float32)
    wfr = Wfin.reshape(KF, P, OUT_F).transpose(1, 0, 2)
    wfh = np.ascontiguousarray(wfr.reshape(P, KF * OUT_F)).astype(bf16)
    return xhs, wh, wfh


def kernel(x, W_chunks, b_chunks, W_final, b_final):
    xhs, wh, wfh = _pack_inputs(x, W_chunks, W_final)
    bct = np.ascontiguousarray(
        np.asarray(b_chunks, dtype=np.float32).T)          # [512, 8]
    bfb = np.ascontiguousarray(np.broadcast_to(
        np.asarray(b_final, dtype=np.float32).reshape(1, OUT_F),
        (P, OUT_F)))

    if "nc" not in _cache:
        _cache["nc"] = _build()
    nc = _cache["nc"]

    in_maps = [{
        "xh": xhs[c],
        "wh": wh,
        "wfh": wfh,
        "bct": bct,
        "bfb": bfb,
    } for c in range(N_CORES)]

    res = run_bass_kernel_spmd(nc, in_maps, core_ids=list(range(N_CORES)))
    kernel.last_result = res
    return np.concatenate(
        [res.results[c]["out_shard"] for c in range(N_CORES)], axis=0)


kernel.last_result = None


# revision 39
# speedup vs baseline: 1.0598x; 1.0598x over previous
"""Trainium2 Bass kernel for BlockChunkedActivityRoutedNet (v3, speculative).

Reference (B=4096, IN_F=4096, 8 chunks of 512, top-2 by mean|x| over the
whole batch, chunk Linears 512->512, concat -> final Linear 1024->4096):

    xr = x.reshape(B, 8, 512)
    activities = mean(|xr|, axis=(0, 2))
    i0, i1 = top2(activities)
    h = concat(xr[:, i0] @ Wc[i0] + bc[i0], xr[:, i1] @ Wc[i1] + bc[i1])
    out = h @ W_final + b_final

Distribution: data-parallel over batch across 8 cores (512 rows each).
Per-chunk |x| partial sums are AllGathered (32B per core) and summed
locally so every core computes identical top-2 routing.

Collective latency (entry skew + mesh floor) is hidden by SPECULATIVELY
running L1 for ALL 8 chunks while the AllGather is in flight.  hT for
every chunk is written to a DRAM table (row c*128+p holds that
partition's 4 d-tiles, 4KB rows); once routing lands, the two selected
chunks' hT is pulled back with 2 indirect-DMA row gathers (row =
sel*128 + p) and only L2 remains on the critical path.

DMA strategy: all SBUF-destined inputs are host-packed into [128, N]
matrices so every load is a single large dma_start with multi-KB
per-partition runs (one dma_start splits across all 16 SDMA engines).
Bulk loads ride the qSync HWDGE ring in priority order (x + first W
chunks first, W_final last); small loads, hT-table writes and output
writes ride the qAct ring so they never queue behind W_final.
"""

import numpy as np
import ml_dtypes

import concourse.bass as bass
import concourse.bass_isa as bass_isa
import concourse.bacc as bacc
import concourse.mybir as mybir
from concourse.tile import TileContext
from concourse.bass_utils import run_bass_kernel_spmd

dt = mybir.dt
P = 128

NUM_CHUNKS = 8
TOP_K = 2
IN_F = 4096
HID_F = 4096
OUT_F = 4096
B = 4096
CIN = IN_F // NUM_CHUNKS      # 512
COUT = HID_F // NUM_CHUNKS    # 512
N_CORES = 8
BS = B // N_CORES             # 512 rows per core

KT = CIN // P                 # 4 k-tiles per chunk (q groups)
DT_ = COUT // P               # 4 d-tiles per chunk
KF = TOP_K * DT_              # 8 k-tiles for the final matmul
OT = OUT_F // 512             # 8 output column tiles of 512
BT = BS // P                  # 4 batch tiles per core

XW = KT * BS                  # 2048: per-chunk x cols [q*512 + b]
WW = KT * COUT                # 2048: per-chunk w cols [q*512 + d]
CH_HEAD = 4                   # chunks whose weights load paired with x

_cache = {}


def _build():
    nc = bacc.Bacc(num_devices=N_CORES, name="chunk_routed_v3",
                   num_swdge_queues=4)

    # host-packed inputs: [128, N] matrices, per-partition-contiguous
    xh = nc.dram_tensor("xh", [P, NUM_CHUNKS * XW], dt.bfloat16,
                        kind="ExternalInput")
    wh = nc.dram_tensor("wh", [P, NUM_CHUNKS * WW], dt.bfloat16,
                        kind="ExternalInput")
    wfh = nc.dram_tensor("wfh", [P, KF * OUT_F], dt.bfloat16,
                         kind="ExternalInput")
    bct = nc.dram_tensor("bct", [COUT, NUM_CHUNKS], dt.float32,
                         kind="ExternalInput")
    bfb = nc.dram_tensor("bfb", [P, OUT_F], dt.float32,
                         kind="ExternalInput")
    out = nc.dram_tensor("out_shard", [BS, OUT_F], dt.float32,
                         kind="ExternalOutput")

    with TileContext(nc) as tc:
        with tc.tile_pool(name="consts", bufs=1) as consts, \
             tc.tile_pool(name="xl", bufs=1) as xl_pool, \
             tc.tile_pool(name="wkp", bufs=1) as wkp, \
             tc.tile_pool(name="wfp", bufs=8) as wfp, \
             tc.tile_pool(name="bias", bufs=1) as biasp, \
             tc.tile_pool(name="route", bufs=1) as route, \
             tc.tile_pool(name="hstage", bufs=3) as hstage, \
             tc.tile_pool(name="hselp", bufs=1) as hselp, \
             tc.tile_pool(name="outs", bufs=8) as outs, \
             tc.tile_pool(name="dram", bufs=1, space="DRAM") as dram:

            # ---------------- constants ----------------
            ones_col = consts.tile([P, 1], dt.float32)
            nc.vector.memset(ones_col[:], 1.0)
            C_row = consts.tile([1, P], dt.float32)
            C_row_i = consts.tile([1, P], dt.int32)
            nc.gpsimd.iota(C_row_i[:], pattern=[[1, P]], base=0,
                           channel_multiplier=0)
            nc.vector.tensor_copy(C_row[:], C_row_i[:])
            C_p = consts.tile([P, 1], dt.int32)
            nc.gpsimd.iota(C_p[:], pattern=[[1, 1]], base=0,
                           channel_multiplier=1)
            C_pf = consts.tile([P, 1], dt.float32)
            nc.vector.tensor_copy(C_pf[:], C_p[:])

            # ---------------- small loads (qAct ring) ----------------
            # b_final pre-broadcast on host: plain 2MB load, no PE/gpsimd
            bfin_bc = biasp.tile([P, OUT_F], dt.float32)
            nc.scalar.dma_start(bfin_bc[:], bfb[:])
            bias_sb = biasp.tile([P, DT_ * NUM_CHUNKS], dt.float32)
            # bias_sb[p, d*8+c] = b_chunks[c, 128d+p]
            for d in range(DT_):
                nc.scalar.dma_start(
                    bias_sb[:, d * NUM_CHUNKS:(d + 1) * NUM_CHUNKS],
                    bct[d * P:(d + 1) * P, :])

            # ---------------- bulk loads (qSync ring, priority FIFO) ----
            xls = [xl_pool.tile([P, XW], dt.bfloat16, tag=f"xl{c}",
                                name=f"xl{c}") for c in range(NUM_CHUNKS)]
            wks = [wkp.tile([P, WW], dt.bfloat16, tag=f"wk{c}",
                            name=f"wk{c}") for c in range(NUM_CHUNKS)]

            for c in range(NUM_CHUNKS):
                nc.sync.dma_start(xls[c][:], xh[:, c * XW:(c + 1) * XW])
            for c in range(NUM_CHUNKS):
                nc.sync.dma_start(wks[c][:], wh[:, c * WW:(c + 1) * WW])
            wf_t = [wfp.tile([P, OUT_F], dt.bfloat16, tag="wf",
                             name=f"wf{kf}") for kf in range(KF)]
            for kf in range(KF):
                nc.sync.dma_start(wf_t[kf][:],
                                  wfh[:, kf * OUT_F:(kf + 1) * OUT_F])

            # ---------------- activities ----------------
            # evens on scalar, odds on DVE (DVE is ~3x faster per reduce;
            # the last-arriving chunk 7 must clear quickly)
            actcol = route.tile([P, NUM_CHUNKS], dt.float32)
            scr = route.tile([P, XW], dt.bfloat16)   # Abs throwaway
            for c in range(NUM_CHUNKS):
                if c % 2 == 1:
                    nc.vector.tensor_reduce(
                        actcol[:, c:c + 1], xls[c][:],
                        axis=mybir.AxisListType.X, op=mybir.AluOpType.add,
                        apply_absolute_value=True)
                else:
                    nc.scalar.activation(
                        scr[:], xls[c][:],
                        mybir.ActivationFunctionType.Abs,
                        accum_out=actcol[:, c:c + 1])

            # collective buffers + hT table
            cc_in = dram.tile([1, NUM_CHUNKS], dt.float32)
            cc_out = dram.tile([1, N_CORES * NUM_CHUNKS], dt.float32)
            # hT table: row c*128+p = chunk c / partition p, cols = (d, b)
            hT_all = dram.tile([NUM_CHUNKS * P, DT_ * BS], dt.bfloat16)

            # ---------------- L1: speculative, all 8 chunks ----------------
            with tc.tile_pool(name="ps_pre", bufs=1, space="PSUM") as ps_pre, \
                 tc.tile_pool(name="ps_l1", bufs=2, space="PSUM") as ps_l1:

                def l1_chunk(c):
                    hsb = hstage.tile([P, DT_ * BS], dt.bfloat16, tag="hsb",
                                      name=f"hsb{c}")
                    for d in range(DT_):
                        ph = ps_l1.tile([P, BS], dt.float32, tag="ph",
                                        name=f"ph{c}_{d}")
                        for q in range(KT):
                            nc.tensor.matmul(
                                ph[:],
                                wks[c][:, q * COUT + d * P:
                                       q * COUT + (d + 1) * P],
                                xls[c][:, q * BS:(q + 1) * BS],
                                start=(q == 0), stop=(q == KT - 1))
                        nc.scalar.activation(
                            hsb[:, d * BS:(d + 1) * BS], ph[:],
                            mybir.ActivationFunctionType.Identity,
                            bias=bias_sb[:, d * NUM_CHUNKS + c:
                                         d * NUM_CHUNKS + c + 1])
                    nc.scalar.dma_start(
                        hT_all[c * P:(c + 1) * P, :], hsb[:])

                # activity partition-reduce + collective trigger first: the
                # PE idles until the chunk weights land anyway, and the
                # trigger must beat the CC-stream rendezvous floor.
                # act_l copy on DVE and cc_in on the (empty) SWDGE ring so
                # the trigger never queues behind evictions or hT writes.
                act_ps = ps_pre.tile([1, NUM_CHUNKS], dt.float32, tag="psa")
                nc.tensor.matmul(act_ps[:], ones_col[:], actcol[:],
                                 start=True, stop=True)
                act_l = route.tile([1, NUM_CHUNKS], dt.float32)
                nc.vector.tensor_copy(act_l[:], act_ps[:])
                nc.gpsimd.dma_start(cc_in[:], act_l[:])
                nc.gpsimd.collective_compute(
                    "AllGather", mybir.AluOpType.bypass,
                    replica_groups=[list(range(N_CORES))],
                    ins=[cc_in.opt()], outs=[cc_out.opt()])

                for c in range(NUM_CHUNKS):
                    l1_chunk(c)

                # ---------------- routing ----------------
                # ag_sb load on gpsimd only: on scalar/sync the Tile
                # scheduler hoists it before the tail L1 evictions and the
                # whole engine then blocks on the collective (priority
                # inversion that stalls L1's PSUM banks ~30us)
                ag_sb = route.tile([1, N_CORES * NUM_CHUNKS], dt.float32)
                nc.gpsimd.dma_start(ag_sb[:], cc_out[:])
                t32 = route.tile([1, 32], dt.float32)
                nc.vector.tensor_tensor(out=t32[:], in0=ag_sb[:, 0:32],
                                        in1=ag_sb[:, 32:64],
                                        op=mybir.AluOpType.add)
                t16 = route.tile([1, 16], dt.float32)
                nc.vector.tensor_tensor(out=t16[:], in0=t32[:, 0:16],
                                        in1=t32[:, 16:32],
                                        op=mybir.AluOpType.add)
                acts8 = route.tile([1, NUM_CHUNKS], dt.float32)
                nc.vector.tensor_tensor(out=acts8[:], in0=t16[:, 0:8],
                                        in1=t16[:, 8:16],
                                        op=mybir.AluOpType.add)
                maxv = route.tile([1, 8], dt.float32)
                nc.vector.max(maxv[:], acts8[:])
                maxi = route.tile([1, 8], dt.uint32)
                nc.vector.max_index(maxi[:], maxv[:], acts8[:])
                maxi_f = route.tile([1, 8], dt.float32)
                nc.vector.tensor_copy(maxi_f[:], maxi[:])

                # broadcast top-2 ids to all partitions (gpsimd; the only
                # later gpsimd work is the gathers, which need routing
                # anyway, so no priority inversion is possible here)
                bc2f = route.tile([P, TOP_K], dt.float32)
                nc.gpsimd.partition_broadcast(bc2f[:], maxi_f[:, 0:TOP_K])
                bc128 = route.tile([P, TOP_K], dt.float32)
                nc.vector.tensor_scalar_mul(bc128[:], bc2f[:], float(P))
                offW_f = route.tile([P, TOP_K], dt.float32)
                for s in range(TOP_K):
                    nc.vector.tensor_scalar(
                        offW_f[:, s:s + 1], C_pf[:], bc128[:, s:s + 1],
                        scalar2=None, op0=mybir.AluOpType.add)
                offW = route.tile([P, TOP_K], dt.int32)
                nc.vector.tensor_copy(offW[:], offW_f[:])

                # gather hT of the two selected chunks: row = sel*128 + p
                hsel = [hselp.tile([P, DT_ * BS], dt.bfloat16, tag=f"hs{s}",
                                   name=f"hs{s}") for s in range(TOP_K)]
                for s in range(TOP_K):
                    nc.gpsimd.indirect_dma_start(
                        out=hsel[s][:], out_offset=None,
                        in_=hT_all[:],
                        in_offset=bass.IndirectOffsetOnAxis(
                            ap=offW[:, s:s + 1], axis=0))

            # ---------------- L2: out = h @ W_final + b_final ----------
            with tc.tile_pool(name="ps_l2", bufs=8, space="PSUM") as ps_l2:
                for bt in range(BT):
                    po = [ps_l2.tile([P, 512], dt.float32, tag="po",
                                     name=f"po{bt}_{o}") for o in range(OT)]
                    for kf in range(KF):
                        s, d = divmod(kf, DT_)
                        lhsT = hsel[s][:, d * BS + bt * P:
                                       d * BS + (bt + 1) * P]
                        for o in range(OT):
                            nc.tensor.matmul(
                                po[o][:], lhsT,
                                wf_t[kf][:, o * 512:(o + 1) * 512],
                                start=(kf == 0), stop=(kf == KF - 1))
                    for o in range(OT):
                        osl = slice(o * 512, (o + 1) * 512)
                        ot_sb = outs.tile([P, 512], dt.float32, tag="ot",
                                          name=f"ot{bt}_{o}")
                        nc.vector.tensor_tensor(
                            out=ot_sb[:], in0=po[o][:], in1=bfin_bc[:, osl],
                            op=mybir.AluOpType.add)
                        # alternate rings: halves the serial write tail
                        eng = nc.scalar if o % 2 == 0 else nc.sync
                        eng.dma_start(
                            out[bt * P:(bt + 1) * P, osl], ot_sb[:])
    nc.compile()
    return nc


def _pack_inputs(x, W_chunks, W_final):
    """Host-side layout packing (pure layout, no computation)."""
    bf16 = ml_dtypes.bfloat16
    # per-core xh: [128, 8*2048], xh[p, c*2048 + q*512 + b] = x[b, 512c+4p+q]
    xhs = []
    for c0 in range(N_CORES):
        xs = np.asarray(x[c0 * BS:(c0 + 1) * BS], dtype=np.float32)  # [512, 4096]
        # [b, c, p, q] -> [p, c, q, b]
        xr = xs.reshape(BS, NUM_CHUNKS, P, KT).transpose(2, 1, 3, 0)
        xhs.append(np.ascontiguousarray(xr.reshape(P, NUM_CHUNKS * XW))
                   .astype(bf16))
    # wh[p, c*2048 + q*512 + d] = W[c, 4p+q, d]
    Wc = np.asarray(W_chunks, dtype=np.float32)
    wr = Wc.reshape(NUM_CHUNKS, P, KT, COUT).transpose(1, 0, 2, 3)
    wh = np.ascontiguousarray(wr.reshape(P, NUM_CHUNKS * WW)).astype(bf16)
    # wfh[p, kf*4096 + n] = W_final[128*kf + p, n]
    Wfin = np.asarray(W_final, dtype=np.float32)
    wfr = Wfin.reshape(KF, P, OUT_F).transpose(1, 0, 2)
    wfh = np.ascontiguousarray(wfr.reshape(P, KF * OUT_F)).astype(bf16)
    return xhs, wh, wfh


def kernel(x, W_chunks, b_chunks, W_final, b_final):
    xhs, wh, wfh = _pack_inputs(x, W_chunks, W_final)
    bct = np.ascontiguousarray(
        np.asarray(b_chunks, dtype=np.float32).T)          # [512, 8]
    bfb = np.ascontiguousarray(np.broadcast_to(
        np.asarray(b_final, dtype=np.float32).reshape(1, OUT_F),
        (P, OUT_F)))

    if "nc" not in _cache:
        _cache["nc"] = _build()
    nc = _cache["nc"]

    in_maps = [{
        "xh": xhs[c],
        "wh": wh,
        "wfh": wfh,
        "bct": bct,
        "bfb": bfb,
    } for c in range(N_CORES)]

    res = run_bass_kernel_spmd(nc, in_maps, core_ids=list(range(N_CORES)))
    kernel.last_result = res
    return np.concatenate(
        [res.results[c]["out_shard"] for c in range(N_CORES)], axis=0)


kernel.last_result = None
